# revision 11
# baseline (speedup 1.0000x reference)
"""Trainium2 Bass kernel for DiversityInjection (MoE-style per-agent low-rank
perturbation + LayerNorm).

Strategy: expert-parallel over the 256 agents. The host routes tokens to the
core that owns their agent (MoE dispatch done host-side), packs them into
fixed-capacity per-agent slots (CAP tokens), and each core runs dense batched
matmuls over groups of G=3 slots (126 tokens per group tile):

  mm1 (3 slots at once): psum1[96, 126] = [U_a|U_b|U_c]^T @ hT3
        8 contract chunks of 128; useful output = 3 diagonal [32, 42] blocks
  mm2 (3 slots at once, block-diag): psum2[126, 512] =
        blockdiag(intT_a, intT_b, intT_c)^T(96x126) @ [V_a; V_b; V_c](96x512)
  out = LayerNorm(h + pert) fused via bn_stats + scalar activation

The padded output is scattered back to original token order on the host.
"""

import os
import sys

for _p in ("/opt/trn_rl_repo", "/root/.axon_site/_ro/trn_rl_repo"):
    if os.path.isdir(_p) and _p not in sys.path:
        sys.path.insert(0, _p)

import numpy as np

N_CORES = 8
CAP = 42           # tokens per slot (per-agent capacity)
G = 3              # slots per group tile (G*CAP <= 128, G*rank <= 128)
ALPHA_MAX = 5.0
LN_EPS = 1e-5
VARIANT = os.environ.get("BASS_KERNEL_VARIANT", "t32")

_PROGRAM_CACHE = {}


def _reference_numpy(h, log_alpha, ln_gamma, ln_beta, projection_u, projection_v,
                     agent_ids):
    """Fallback pure-numpy implementation (used only if packing does not fit)."""
    num_agents = projection_u.shape[0]
    ids = agent_ids % num_agents
    alpha = min(np.exp(np.float32(log_alpha)), np.float32(ALPHA_MAX))
    out = np.empty_like(h)
    for a in range(num_agents):
        m = ids == a
        if not m.any():
            continue
        hb = h[m]
        pert = (hb @ projection_u[a]) @ projection_v[a]
        out[m] = hb + alpha * pert
    mean = out.mean(-1, keepdims=True, dtype=np.float64)
    var = out.var(-1, keepdims=True, dtype=np.float64)
    out = (out - mean) / np.sqrt(var + LN_EPS)
    return (out * ln_gamma + ln_beta).astype(h.dtype)


def _build_program(nslot, hidden, rank, variant):
    """Build the per-core Bass program. Same program runs SPMD on all 8 cores."""
    from contextlib import ExitStack

    import concourse.bacc as bacc
    import concourse.mybir as mybir
    import concourse.tile as tile

    assert hidden == 1024 and rank == 32
    assert nslot % G == 0
    ngroup = nslot // G
    nchunk = hidden // 128
    T = G * CAP          # tokens per group tile (126)
    KR = G * rank        # stacked rank (96)

    mmdt = mybir.dt.float32r if variant.endswith("r") else mybir.dt.float32

    nc = bacc.Bacc("TRN2", target_bir_lowering=False, debug=False)

    u_d = nc.dram_tensor("u_sw", (ngroup, 128, nchunk * KR), mmdt,
                         kind="ExternalInput")
    v_d = nc.dram_tensor("v_sw", (ngroup, KR, hidden), mmdt,
                         kind="ExternalInput")
    hT_d = nc.dram_tensor("hT_sw", (ngroup, 128, nchunk * T), mmdt,
                          kind="ExternalInput")
    id_d = nc.dram_tensor("ident", (128, 128), mybir.dt.float32,
                          kind="ExternalInput")
    out_d = nc.dram_tensor("out_pk", (ngroup, T, hidden), mybir.dt.float32,
                           kind="ExternalOutput")

    with tile.TileContext(nc) as tc, ExitStack() as ctx:
        upool = ctx.enter_context(tc.tile_pool(name="u", bufs=6))
        vpool = ctx.enter_context(tc.tile_pool(name="v", bufs=6))
        htpool = ctx.enter_context(tc.tile_pool(name="hT", bufs=6))
        bpool = ctx.enter_context(tc.tile_pool(name="blk", bufs=3))
        spool = ctx.enter_context(tc.tile_pool(name="stats", bufs=8))
        opool = ctx.enter_context(tc.tile_pool(name="o", bufs=6))
        cpool = ctx.enter_context(tc.tile_pool(name="const", bufs=1))
        p1pool = ctx.enter_context(tc.tile_pool(name="psum1", bufs=4, space="PSUM"))
        p2pool = ctx.enter_context(tc.tile_pool(name="psum2", bufs=2, space="PSUM"))

        eps_t = cpool.tile([128, 1], mybir.dt.float32)
        nc.vector.memset(eps_t[:], LN_EPS)
        id_t = cpool.tile([128, 128], mybir.dt.float32)
        nc.gpsimd.dma_start(id_t[:], id_d[:])

        for g in range(ngroup):
            u_t = upool.tile([128, nchunk * KR], mmdt)
            hc = nchunk // 2
            nc.scalar.dma_start(u_t[:, 0:KR], u_d[g][:, 0:KR])
            nc.scalar.dma_start(u_t[:, KR:hc * KR], u_d[g][:, KR:hc * KR])
            nc.scalar.dma_start(u_t[:, hc * KR:], u_d[g][:, hc * KR:])
            hT_t = htpool.tile([128, nchunk * T], mmdt)
            nc.sync.dma_start(hT_t[:, 0:T], hT_d[g][:, 0:T])
            nc.sync.dma_start(hT_t[:, T:hc * T], hT_d[g][:, T:hc * T])
            nc.sync.dma_start(hT_t[:, hc * T:], hT_d[g][:, hc * T:])

            psum1 = p1pool.tile([KR, T], mybir.dt.float32)
            for c in range(nchunk):
                nc.tensor.matmul(
                    psum1[:],
                    u_t[:, c * KR:(c + 1) * KR],
                    hT_t[:, c * T:(c + 1) * T],
                    start=(c == 0), stop=(c == nchunk - 1),
                )

            # block-diag [KR, T] lhsT: diagonal [rank, CAP] blocks from psum1
            blk = bpool.tile([KR, T], mmdt)
            for s in range(G):
                for s2 in range(G):
                    if s == s2:
                        nc.vector.tensor_copy(
                            blk[s * rank:(s + 1) * rank,
                                s2 * CAP:(s2 + 1) * CAP],
                            psum1[s * rank:(s + 1) * rank,
                                  s2 * CAP:(s2 + 1) * CAP])
                    else:
                        nc.gpsimd.memset(
                            blk[s * rank:(s + 1) * rank,
                                s2 * CAP:(s2 + 1) * CAP], 0.0)

            v_t = vpool.tile([KR, hidden], mmdt)
            nc.sync.dma_start(v_t[:], v_d[g])

            psum2 = p2pool.tile([128, hidden], mybir.dt.float32)
            for q in range(hidden // 512):
                nc.tensor.matmul(
                    psum2[0:T, q * 512:(q + 1) * 512],
                    blk[:],
                    v_t[:, q * 512:(q + 1) * 512],
                    start=True, stop=True,
                )
            # accumulate the residual h into psum2 by transposing hT chunks
            # through the PE (x = h + pert materializes in PSUM, no h reload)
            for c in range(nchunk):
                nc.tensor.matmul(
                    psum2[0:T, c * 128:(c + 1) * 128],
                    hT_t[:, c * T:(c + 1) * T],
                    id_t[:],
                    is_transpose=True, start=False, stop=True,
                    skip_group_check=True,
                )

            stats = spool.tile([128, 6 * (hidden // 512)], mybir.dt.float32)
            for q in range(hidden // 512):
                nc.vector.bn_stats(stats[0:T, q * 6:(q + 1) * 6],
                                   psum2[0:T, q * 512:(q + 1) * 512])
            aggr = spool.tile([128, 2], mybir.dt.float32)
            nc.vector.bn_aggr(aggr[0:T, :],
                              stats[0:T, :].rearrange("p (c s) -> p c s", s=3))
            std = spool.tile([128, 1], mybir.dt.float32)
            nc.scalar.activation(std[0:T, :], aggr[0:T, 1:2],
                                 mybir.ActivationFunctionType.Sqrt,
                                 bias=eps_t[0:T, 0:1])
            rstd = spool.tile([128, 1], mybir.dt.float32)
            nc.vector.reciprocal(rstd[0:T, :], std[0:T, :])
            nmr = spool.tile([128, 1], mybir.dt.float32)
            nc.vector.scalar_tensor_tensor(nmr[0:T, :], aggr[0:T, 0:1], -1.0,
                                           rstd[0:T, :],
                                           mybir.AluOpType.mult,
                                           mybir.AluOpType.mult)
            o_t = opool.tile([128, hidden], mybir.dt.float32)
            nc.scalar.activation(o_t[0:T, :], psum2[0:T, :],
                                 mybir.ActivationFunctionType.Identity,
                                 bias=nmr[0:T, 0:1], scale=rstd[0:T, 0:1])
            if g % 2 == 0:
                nc.gpsimd.dma_start(out_d[g], o_t[0:T, :])
            else:
                nc.sync.dma_start(out_d[g], o_t[0:T, :])

    nc.finalize()
    return nc


def _get_program(nslot, hidden, rank, variant):
    key = (nslot, hidden, rank, variant)
    if key not in _PROGRAM_CACHE:
        _PROGRAM_CACHE[key] = _build_program(nslot, hidden, rank, variant)
    return _PROGRAM_CACHE[key]


def kernel(h, log_alpha, ln_gamma, ln_beta, projection_u, projection_v,
           agent_ids):
    h = np.asarray(h, dtype=np.float32)
    projection_u = np.asarray(projection_u, dtype=np.float32)
    projection_v = np.asarray(projection_v, dtype=np.float32)
    ln_gamma = np.asarray(ln_gamma, dtype=np.float32)
    ln_beta = np.asarray(ln_beta, dtype=np.float32)
    ids_raw = np.asarray(agent_ids)
    log_alpha = np.float32(np.asarray(log_alpha))

    B, H = h.shape
    A, H2, R = projection_u.shape
    ids = (ids_raw.astype(np.int64) % A).astype(np.int32)

    if H != 1024 or H2 != H or R != 32 or projection_v.shape != (A, R, H):
        return _reference_numpy(h, log_alpha, ln_gamma, ln_beta, projection_u,
                                projection_v, agent_ids)

    alpha = np.float32(min(np.exp(log_alpha), np.float32(ALPHA_MAX)))
    use_gamma = not np.all(ln_gamma == 1.0)
    use_beta = not np.all(ln_beta == 0.0)

    # ---- host-side MoE dispatch: sort tokens by agent, build capacity slots
    order = np.argsort(ids, kind="stable").astype(np.int64)
    counts = np.bincount(ids, minlength=A)
    starts = np.zeros(A + 1, np.int64)
    np.cumsum(counts, out=starts[1:])

    slot_agent = []   # agent id per slot
    slot_rows = []    # (start, n) into `order` per slot
    for a in range(A):
        n = int(counts[a])
        s = int(starts[a])
        while n > 0:
            take = min(n, CAP)
            slot_agent.append(a)
            slot_rows.append((s, take))
            s += take
            n -= take
    total_slots = len(slot_agent)
    nslot = -(-total_slots // N_CORES)
    nslot = max(nslot, G)
    if nslot % G:
        nslot += G - nslot % G
    if nslot > 96:  # way off the expected distribution; play it safe
        return _reference_numpy(h, log_alpha, ln_gamma, ln_beta, projection_u,
                                projection_v, agent_ids)
    while len(slot_agent) < nslot * N_CORES:
        slot_agent.append(0)
        slot_rows.append((0, 0))
    slot_agent = np.asarray(slot_agent, np.int64)

    ngroup = nslot // G
    nchunk = H // 128
    T = G * CAP
    KR = G * R

    # row_idx: global token index feeding each padded row (clamped for pads)
    nrows = nslot * CAP
    row_idx = np.zeros((N_CORES, nrows), np.int64)
    row_valid = np.zeros((N_CORES, nrows), bool)
    for j, (s, n) in enumerate(slot_rows):
        core, sl = divmod(j, nslot)
        r0 = sl * CAP
        if n:
            row_idx[core, r0:r0 + n] = order[s:s + n]
            row_valid[core, r0:r0 + n] = True

    h_pk = h[row_idx].reshape(N_CORES, ngroup, T, H)
    # hT per group: [p(128), c(8), t(T)]
    hT_sw = np.ascontiguousarray(
        h_pk.reshape(N_CORES, ngroup, T, nchunk, 128)
        .transpose(0, 1, 4, 3, 2)).reshape(N_CORES, ngroup, 128, nchunk * T)
    ident = np.eye(128, dtype=np.float32)

    sa = slot_agent.reshape(N_CORES, nslot)
    # u: [g, p(128), c(8), s(G), r(32)]
    u_sw = np.ascontiguousarray(
        projection_u[sa]                                  # [8, ns, H, R]
        .reshape(N_CORES, ngroup, G, nchunk, 128, R)
        .transpose(0, 1, 4, 3, 2, 5)                      # [8, g, 128, c, G, R]
    ).reshape(N_CORES, ngroup, 128, nchunk * KR)
    v_sw = np.ascontiguousarray(alpha * projection_v[sa]).reshape(
        N_CORES, ngroup, KR, H)

    in_maps = []
    for core in range(N_CORES):
        m = {
            "u_sw": u_sw[core],
            "v_sw": v_sw[core],
            "hT_sw": hT_sw[core],
            "ident": ident,
        }
        in_maps.append(m)

    nc = _get_program(nslot, H, R, VARIANT)

    from concourse.bass_utils import run_bass_kernel_spmd
    res = run_bass_kernel_spmd(nc, in_maps, list(range(N_CORES)))

    out = np.empty_like(h)
    for core in range(N_CORES):
        o = np.asarray(res.results[core]["out_pk"]).reshape(nrows, H)
        out[row_idx[core][row_valid[core]]] = o[row_valid[core]]
    # gamma/beta are applied host-side (the device computes plain LayerNorm);
    # for the common gamma=1/beta=0 case this is a no-op.
    if use_gamma:
        out *= ln_gamma
    if use_beta:
        out += ln_beta
    return out


# revision 12
# speedup vs baseline: 1.0835x; 1.0835x over previous
"""Trainium2 Bass kernel for DiversityInjection (MoE-style per-agent low-rank
perturbation + LayerNorm).

Strategy: expert-parallel over the 256 agents. The host routes tokens to the
core that owns their agent (MoE dispatch done host-side), packs them into
fixed-capacity per-agent slots (CAP tokens), and each core runs dense batched
matmuls over groups of G=3 slots (126 tokens per group tile):

  mm1 (3 slots at once): psum1[96, 126] = [U_a|U_b|U_c]^T @ hT3
        8 contract chunks of 128; useful output = 3 diagonal [32, 42] blocks
  mm2 (3 slots at once, block-diag): psum2[126, 512] =
        blockdiag(intT_a, intT_b, intT_c)^T(96x126) @ [V_a; V_b; V_c](96x512)
  out = LayerNorm(h + pert) fused via bn_stats + scalar activation

The padded output is scattered back to original token order on the host.
"""

import os
import sys

for _p in ("/opt/trn_rl_repo", "/root/.axon_site/_ro/trn_rl_repo"):
    if os.path.isdir(_p) and _p not in sys.path:
        sys.path.insert(0, _p)

import numpy as np

N_CORES = 8
CAP = 42           # tokens per slot (per-agent capacity)
G = 3              # slots per group tile (G*CAP <= 128, G*rank <= 128)
ALPHA_MAX = 5.0
LN_EPS = 1e-5
VARIANT = os.environ.get("BASS_KERNEL_VARIANT", "t32")

_PROGRAM_CACHE = {}


def _reference_numpy(h, log_alpha, ln_gamma, ln_beta, projection_u, projection_v,
                     agent_ids):
    """Fallback pure-numpy implementation (used only if packing does not fit)."""
    num_agents = projection_u.shape[0]
    ids = agent_ids % num_agents
    alpha = min(np.exp(np.float32(log_alpha)), np.float32(ALPHA_MAX))
    out = np.empty_like(h)
    for a in range(num_agents):
        m = ids == a
        if not m.any():
            continue
        hb = h[m]
        pert = (hb @ projection_u[a]) @ projection_v[a]
        out[m] = hb + alpha * pert
    mean = out.mean(-1, keepdims=True, dtype=np.float64)
    var = out.var(-1, keepdims=True, dtype=np.float64)
    out = (out - mean) / np.sqrt(var + LN_EPS)
    return (out * ln_gamma + ln_beta).astype(h.dtype)


def _build_program(nslot, hidden, rank, variant):
    """Build the per-core Bass program. Same program runs SPMD on all 8 cores."""
    from contextlib import ExitStack

    import concourse.bacc as bacc
    import concourse.mybir as mybir
    import concourse.tile as tile

    assert hidden == 1024 and rank == 32
    assert nslot % G == 0
    ngroup = nslot // G
    nchunk = hidden // 128
    T = G * CAP          # tokens per group tile (126)
    KR = G * rank        # stacked rank (96)

    mmdt = mybir.dt.float32r if variant.endswith("r") else mybir.dt.float32

    nc = bacc.Bacc("TRN2", target_bir_lowering=False, debug=False)

    u_d = nc.dram_tensor("u_sw", (ngroup, 128, nchunk * KR), mmdt,
                         kind="ExternalInput")
    v_d = nc.dram_tensor("v_sw", (ngroup, KR, hidden), mmdt,
                         kind="ExternalInput")
    hT_d = nc.dram_tensor("hT_sw", (ngroup, 128, nchunk * T), mmdt,
                          kind="ExternalInput")
    id_d = nc.dram_tensor("ident", (128, 128), mybir.dt.float32,
                          kind="ExternalInput")
    out_d = nc.dram_tensor("out_pk", (ngroup, T, hidden), mybir.dt.float32,
                           kind="ExternalOutput")

    with tile.TileContext(nc) as tc, ExitStack() as ctx:
        upool = ctx.enter_context(tc.tile_pool(name="u", bufs=6))
        vpool = ctx.enter_context(tc.tile_pool(name="v", bufs=6))
        htpool = ctx.enter_context(tc.tile_pool(name="hT", bufs=6))
        bpool = ctx.enter_context(tc.tile_pool(name="blk", bufs=3))
        spool = ctx.enter_context(tc.tile_pool(name="stats", bufs=8))
        opool = ctx.enter_context(tc.tile_pool(name="o", bufs=6))
        cpool = ctx.enter_context(tc.tile_pool(name="const", bufs=1))
        p1pool = ctx.enter_context(tc.tile_pool(name="psum1", bufs=4, space="PSUM"))
        p2pool = ctx.enter_context(tc.tile_pool(name="psum2", bufs=2, space="PSUM"))

        eps_t = cpool.tile([128, 1], mybir.dt.float32)
        nc.vector.memset(eps_t[:], LN_EPS)
        id_t = cpool.tile([128, 128], mybir.dt.float32)
        nc.gpsimd.dma_start(id_t[:], id_d[:])

        for g in range(ngroup):
            u_t = upool.tile([128, nchunk * KR], mmdt)
            hc = nchunk // 2
            nc.scalar.dma_start(u_t[:, 0:hc * KR], u_d[g][:, 0:hc * KR])
            nc.scalar.dma_start(u_t[:, hc * KR:], u_d[g][:, hc * KR:])
            hT_t = htpool.tile([128, nchunk * T], mmdt)
            nc.sync.dma_start(hT_t[:, 0:hc * T], hT_d[g][:, 0:hc * T])
            nc.sync.dma_start(hT_t[:, hc * T:], hT_d[g][:, hc * T:])

            psum1 = p1pool.tile([KR, T], mybir.dt.float32)
            for c in range(nchunk):
                nc.tensor.matmul(
                    psum1[:],
                    u_t[:, c * KR:(c + 1) * KR],
                    hT_t[:, c * T:(c + 1) * T],
                    start=(c == 0), stop=(c == nchunk - 1),
                )

            # block-diag [KR, T] lhsT: diagonal [rank, CAP] blocks from psum1
            blk = bpool.tile([KR, T], mmdt)
            for s in range(G):
                for s2 in range(G):
                    if s == s2:
                        nc.vector.tensor_copy(
                            blk[s * rank:(s + 1) * rank,
                                s2 * CAP:(s2 + 1) * CAP],
                            psum1[s * rank:(s + 1) * rank,
                                  s2 * CAP:(s2 + 1) * CAP])
                    else:
                        nc.gpsimd.memset(
                            blk[s * rank:(s + 1) * rank,
                                s2 * CAP:(s2 + 1) * CAP], 0.0)

            v_t = vpool.tile([KR, hidden], mmdt)
            nc.sync.dma_start(v_t[:], v_d[g])

            psum2 = p2pool.tile([128, hidden], mybir.dt.float32)
            for q in range(hidden // 512):
                nc.tensor.matmul(
                    psum2[0:T, q * 512:(q + 1) * 512],
                    blk[:],
                    v_t[:, q * 512:(q + 1) * 512],
                    start=True, stop=True,
                )
            # accumulate the residual h into psum2 by transposing hT chunks
            # through the PE (x = h + pert materializes in PSUM, no h reload)
            for c in range(nchunk):
                nc.tensor.matmul(
                    psum2[0:T, c * 128:(c + 1) * 128],
                    hT_t[:, c * T:(c + 1) * T],
                    id_t[:],
                    is_transpose=True, start=False, stop=True,
                    skip_group_check=True,
                )

            stats = spool.tile([128, 6 * (hidden // 512)], mybir.dt.float32)
            for q in range(hidden // 512):
                nc.vector.bn_stats(stats[0:T, q * 6:(q + 1) * 6],
                                   psum2[0:T, q * 512:(q + 1) * 512])
            aggr = spool.tile([128, 2], mybir.dt.float32)
            nc.vector.bn_aggr(aggr[0:T, :],
                              stats[0:T, :].rearrange("p (c s) -> p c s", s=3))
            std = spool.tile([128, 1], mybir.dt.float32)
            nc.scalar.activation(std[0:T, :], aggr[0:T, 1:2],
                                 mybir.ActivationFunctionType.Sqrt,
                                 bias=eps_t[0:T, 0:1])
            rstd = spool.tile([128, 1], mybir.dt.float32)
            nc.vector.reciprocal(rstd[0:T, :], std[0:T, :])
            nmr = spool.tile([128, 1], mybir.dt.float32)
            nc.vector.scalar_tensor_tensor(nmr[0:T, :], aggr[0:T, 0:1], -1.0,
                                           rstd[0:T, :],
                                           mybir.AluOpType.mult,
                                           mybir.AluOpType.mult)
            o_t = opool.tile([128, hidden], mybir.dt.float32)
            nc.scalar.activation(o_t[0:T, :], psum2[0:T, :],
                                 mybir.ActivationFunctionType.Identity,
                                 bias=nmr[0:T, 0:1], scale=rstd[0:T, 0:1])
            if g % 2 == 0:
                nc.gpsimd.dma_start(out_d[g], o_t[0:T, :])
            else:
                nc.sync.dma_start(out_d[g], o_t[0:T, :])

    nc.finalize()
    return nc


def _get_program(nslot, hidden, rank, variant):
    key = (nslot, hidden, rank, variant)
    if key not in _PROGRAM_CACHE:
        _PROGRAM_CACHE[key] = _build_program(nslot, hidden, rank, variant)
    return _PROGRAM_CACHE[key]


def kernel(h, log_alpha, ln_gamma, ln_beta, projection_u, projection_v,
           agent_ids):
    h = np.asarray(h, dtype=np.float32)
    projection_u = np.asarray(projection_u, dtype=np.float32)
    projection_v = np.asarray(projection_v, dtype=np.float32)
    ln_gamma = np.asarray(ln_gamma, dtype=np.float32)
    ln_beta = np.asarray(ln_beta, dtype=np.float32)
    ids_raw = np.asarray(agent_ids)
    log_alpha = np.float32(np.asarray(log_alpha))

    B, H = h.shape
    A, H2, R = projection_u.shape
    ids = (ids_raw.astype(np.int64) % A).astype(np.int32)

    if H != 1024 or H2 != H or R != 32 or projection_v.shape != (A, R, H):
        return _reference_numpy(h, log_alpha, ln_gamma, ln_beta, projection_u,
                                projection_v, agent_ids)

    alpha = np.float32(min(np.exp(log_alpha), np.float32(ALPHA_MAX)))
    use_gamma = not np.all(ln_gamma == 1.0)
    use_beta = not np.all(ln_beta == 0.0)

    # ---- host-side MoE dispatch: sort tokens by agent, build capacity slots
    order = np.argsort(ids, kind="stable").astype(np.int64)
    counts = np.bincount(ids, minlength=A)
    starts = np.zeros(A + 1, np.int64)
    np.cumsum(counts, out=starts[1:])

    slot_agent = []   # agent id per slot
    slot_rows = []    # (start, n) into `order` per slot
    for a in range(A):
        n = int(counts[a])
        s = int(starts[a])
        while n > 0:
            take = min(n, CAP)
            slot_agent.append(a)
            slot_rows.append((s, take))
            s += take
            n -= take
    total_slots = len(slot_agent)
    nslot = -(-total_slots // N_CORES)
    nslot = max(nslot, G)
    if nslot % G:
        nslot += G - nslot % G
    if nslot > 96:  # way off the expected distribution; play it safe
        return _reference_numpy(h, log_alpha, ln_gamma, ln_beta, projection_u,
                                projection_v, agent_ids)
    while len(slot_agent) < nslot * N_CORES:
        slot_agent.append(0)
        slot_rows.append((0, 0))
    slot_agent = np.asarray(slot_agent, np.int64)

    ngroup = nslot // G
    nchunk = H // 128
    T = G * CAP
    KR = G * R

    # row_idx: global token index feeding each padded row (clamped for pads)
    nrows = nslot * CAP
    row_idx = np.zeros((N_CORES, nrows), np.int64)
    row_valid = np.zeros((N_CORES, nrows), bool)
    for j, (s, n) in enumerate(slot_rows):
        core, sl = divmod(j, nslot)
        r0 = sl * CAP
        if n:
            row_idx[core, r0:r0 + n] = order[s:s + n]
            row_valid[core, r0:r0 + n] = True

    h_pk = h[row_idx].reshape(N_CORES, ngroup, T, H)
    # hT per group: [p(128), c(8), t(T)]
    hT_sw = np.ascontiguousarray(
        h_pk.reshape(N_CORES, ngroup, T, nchunk, 128)
        .transpose(0, 1, 4, 3, 2)).reshape(N_CORES, ngroup, 128, nchunk * T)
    ident = np.eye(128, dtype=np.float32)

    sa = slot_agent.reshape(N_CORES, nslot)
    # u: [g, p(128), c(8), s(G), r(32)]
    u_sw = np.ascontiguousarray(
        projection_u[sa]                                  # [8, ns, H, R]
        .reshape(N_CORES, ngroup, G, nchunk, 128, R)
        .transpose(0, 1, 4, 3, 2, 5)                      # [8, g, 128, c, G, R]
    ).reshape(N_CORES, ngroup, 128, nchunk * KR)
    v_sw = np.ascontiguousarray(alpha * projection_v[sa]).reshape(
        N_CORES, ngroup, KR, H)

    in_maps = []
    for core in range(N_CORES):
        m = {
            "u_sw": u_sw[core],
            "v_sw": v_sw[core],
            "hT_sw": hT_sw[core],
            "ident": ident,
        }
        in_maps.append(m)

    nc = _get_program(nslot, H, R, VARIANT)

    from concourse.bass_utils import run_bass_kernel_spmd
    res = run_bass_kernel_spmd(nc, in_maps, list(range(N_CORES)))

    out = np.empty_like(h)
    for core in range(N_CORES):
        o = np.asarray(res.results[core]["out_pk"]).reshape(nrows, H)
        out[row_idx[core][row_valid[core]]] = o[row_valid[core]]
    # gamma/beta are applied host-side (the device computes plain LayerNorm);
    # for the common gamma=1/beta=0 case this is a no-op.
    if use_gamma:
        out *= ln_gamma
    if use_beta:
        out += ln_beta
    return out
